# revision 1
# baseline (speedup 1.0000x reference)
"""
Trainium2 Bass kernel for nn_CrossAttention_62027917689453.

Math (per batch b):
    q = rgb @ Wq                       (N, E)
    k = freq @ Wk                      (N, E)
    scores = q @ k.T / sqrt(E)         (N, N)
    attn = softmax(scores, axis=-1)
    attn_out = attn @ freq             (N, D)
    out = concat([rgb, 0.5 * attn_out], axis=-1)   (N, 2D)

(ifreq / Wv are dead inputs in the reference and are ignored.)

Sharding: data-parallel over batch — 8 batches onto 8 NeuronCores, one
independent (N, N) attention slab per core. Full inputs in, full output out.

Per-core kernel layout choices:
  - All matmuls contract over the partition dim, so activations are needed
    transposed (d on partitions).  rgbT / freqT blocks are produced with PE
    transposes (bf16, 1 cyc/row) against an identity matrix.
  - Scores are computed TRANSPOSED: sT[m, n] = sum_e kT[e,m]^T qT[e,n], which
    makes exp(sT) (layout [m, n]) directly usable as the stationary operand of
    the attention-output matmul U[n, d] = sum_m P[m,n]^T freq[m,d] with freq in
    its natural layout — no transposes of the (N, N) attention matrix.
  - Softmax skips max-subtraction (scores are O(5) for this problem's
    distribution — exp is safe in fp32) and the denominator is obtained with
    N=1 matmuls against a ones-vector, folded into the same PSUM accumulation
    loop; normalization multiplies by 0.5 * reciprocal(colsum) on VectorE.
  - Matmul operands are bf16 (fp32 PSUM accumulation).
"""

import numpy as np

import concourse.bass as bass
import concourse.mybir as mybir
import concourse.tile as tile
from concourse.tile import TileContext

from concourse.masks import make_identity

F32 = mybir.dt.float32
BF16 = mybir.dt.bfloat16

B = 8          # batches == cores
N = 2048       # sequence length (n and m)
D = 1024       # feature dim (d and e)
P = 128        # partitions
NT = N // P    # 16  row chunks
DC = D // P    # 8   feature chunks
NBLK = 512     # n-block width for the q/scores pipeline
NG = N // NBLK # 4   n-blocks
SUB = NBLK // P  # 4 row-chunks per n-block


def _split_multi_waits(nc: bass.Bass) -> int:
    """The walrus build in this container cannot encode multi-semaphore waits
    on several instruction structs (CTRL Drain, PSEUDO_DMA_DIRECT2D, ...):
    setupSyncWait throws an internal error.  Rewrite every instruction that
    carries more than one wait so the extra waits sit on standalone
    single-wait EventSemaphore instructions immediately before it."""
    n_split = 0
    for f in nc.m.functions:
        for blk in f.blocks:
            insts = blk.instructions
            new: list = []
            changed = False
            for inst in insts:
                si = inst.sync_info
                if si is not None and len(si.on_wait) > 1:
                    waits = list(si.on_wait)
                    for w in waits[:-1]:
                        n_split += 1
                        ev = mybir.InstEventSemaphore(
                            name=f"I-msw-{n_split}",
                            ins=[],
                            outs=[],
                            sync_info=mybir.SyncInfo(on_wait=[w], on_update=[]),
                        )
                        ev.engine = inst.engine
                        new.append(ev)
                    si.on_wait.clear()
                    si.on_wait.append(waits[-1])
                    changed = True
                new.append(inst)
            if changed:
                insts[:] = new
    return n_split


def build_program() -> bass.Bass:
    nc = bass.Bass()
    rgb = nc.declare_dram_parameter("rgb", [N, D], F32, isOutput=False)
    freq = nc.declare_dram_parameter("freq", [N, D], F32, isOutput=False)
    wq = nc.declare_dram_parameter("Wq", [D, D], F32, isOutput=False)
    wk = nc.declare_dram_parameter("Wk", [D, D], F32, isOutput=False)
    out = nc.declare_dram_parameter("out", [N, 2 * D], F32, isOutput=True)

    with TileContext(nc) as tc:
        with (
            tc.tile_pool(name="statics", bufs=1) as statics,
            tc.tile_pool(name="ld", bufs=4) as ldp,
            tc.tile_pool(name="bfp", bufs=2) as bfp,
            tc.tile_pool(name="col", bufs=2) as colp,
            tc.tile_pool(name="qtp", bufs=2) as qtp,
            tc.tile_pool(name="pblk", bufs=2) as pblkp,
            tc.tile_pool(name="outp", bufs=3) as outp,
            tc.tile_pool(name="small", bufs=8) as smallp,
            tc.tile_pool(name="ps", bufs=4, space="PSUM") as psp,
            tc.tile_pool(name="psu", bufs=2, space="PSUM") as psup,
        ):
            ident = statics.tile([P, P], BF16, tag="ident")
            make_identity(nc, ident)
            ones = statics.tile([P, 1], BF16, tag="ones")
            nc.vector.memset(ones, 1.0)

            wq_bf = statics.tile([P, DC, D], BF16, tag="wq")
            wk_bf = statics.tile([P, DC, D], BF16, tag="wk")
            freq_bf = statics.tile([P, NT, D], BF16, tag="freqbf")

            # DMA issue order is the critical-path order: the first PE work
            # (freqT transposes) needs the early freq chunks; kT needs Wk;
            # qT of block 0 needs rgb block 0 + Wq; remaining rgb blocks
            # stream inside the main loop.
            def load_freq(mc):
                t = ldp.tile([P, D], F32, tag="ld")
                nc.sync.dma_start(out=t, in_=freq[mc * P:(mc + 1) * P, :])
                nc.vector.tensor_copy(out=freq_bf[:, mc, :], in_=t)

            def load_wk(dc):
                t2 = ldp.tile([P, D], F32, tag="ld")
                nc.sync.dma_start(out=t2, in_=wk[dc * P:(dc + 1) * P, :])
                nc.vector.tensor_copy(out=wk_bf[:, dc, :], in_=t2)

            def load_rgb_group(ng, defer_passthrough=False):
                # load rgb chunks; write the rgb passthrough output half
                rgb_bf = bfp.tile([P, SUB, D], BF16, tag="rgbbf",
                                  name=f"rgb_bf_{ng}")
                fp32_chunks = []
                for s in range(SUB):
                    nchunk = ng * SUB + s
                    t = ldp.tile([P, D], F32, tag="ld")
                    nc.sync.dma_start(
                        out=t, in_=rgb[nchunk * P:(nchunk + 1) * P, :]
                    )
                    nc.vector.tensor_copy(out=rgb_bf[:, s, :], in_=t)
                    if defer_passthrough:
                        fp32_chunks.append(t)
                    else:
                        nc.sync.dma_start(
                            out=out[nchunk * P:(nchunk + 1) * P, 0:D], in_=t
                        )
                return rgb_bf, fp32_chunks

            # DMA issue order is the critical-path order: early freq chunks
            # feed the PE transposes; Wk follows for the dc-outer kT
            # accumulation; rgb block 0 and Wq stream after.
            for mc in range(4):
                load_freq(mc)
            for dc in range(DC):
                load_wk(dc)
            for mc in range(4, NT):
                load_freq(mc)
            rgb_bf0, rgb0_chunks = load_rgb_group(0, defer_passthrough=True)
            for dc in range(DC):
                t = ldp.tile([P, D], F32, tag="ld")
                nc.sync.dma_start(out=t, in_=wq[dc * P:(dc + 1) * P, :])
                nc.vector.tensor_copy(out=wq_bf[:, dc, :], in_=t)

            # ng=0 passthrough writes issue after the critical-path loads
            for s, t in enumerate(rgb0_chunks):
                nc.sync.dma_start(out=out[s * P:(s + 1) * P, 0:D], in_=t)

            # --- kT[e, m] = Wk[d, e]^T  freqT[d, m]  (all m up front) ---
            # Emission order software-pipelines PE work: transposes of group
            # mg+1 are emitted before the kT matmuls of group mg, so the PE
            # has transpose work while Wk is still loading.
            kt_bf = statics.tile([P, DC, N], BF16, tag="kt")
            fcols = [None] * NG

            def emit_ft(mg):
                fcol = colp.tile([P, DC, NBLK], BF16, tag="col")
                for dc in range(DC):
                    ps_t = psp.tile([P, NBLK], BF16, tag="ps")
                    for s in range(SUB):
                        mc = mg * SUB + s
                        nc.tensor.transpose(
                            ps_t[:, s * P:(s + 1) * P],
                            freq_bf[:, mc, dc * P:(dc + 1) * P],
                            ident,
                        )
                    nc.vector.tensor_copy(out=fcol[:, dc, :], in_=ps_t)
                fcols[mg] = fcol

            def emit_kt(mg):
                # dc-outer accumulation: all 8 PSUM banks hold one et-tile
                # accumulator each, so kT matmuls start as soon as wk[0] is
                # resident instead of waiting for all of Wk.  The 8
                # accumulators borrow both PSUM pools (2x [P,1024] + 4x
                # [P,512]).
                fcol = fcols[mg]
                acc_a = psup.tile([P, D], F32, tag="psu")
                acc_b = psup.tile([P, D], F32, tag="psu")
                accs = [
                    acc_a[:, 0:NBLK], acc_a[:, NBLK:D],
                    acc_b[:, 0:NBLK], acc_b[:, NBLK:D],
                ] + [
                    psp.tile([P, NBLK], F32, tag="ps", name=f"kt_acc_{mg}_{j}")
                    for j in range(4)
                ]
                for dc in range(DC):
                    for et in range(DC):
                        nc.tensor.matmul(
                            accs[et],
                            wk_bf[:, dc, et * P:(et + 1) * P],
                            fcol[:, dc, :],
                            start=(dc == 0),
                            stop=(dc == DC - 1),
                        )
                for et in range(DC):
                    dst = kt_bf[:, et, mg * NBLK:(mg + 1) * NBLK]
                    if et % 2 == 0:
                        nc.scalar.copy(out=dst, in_=accs[et])
                    else:
                        nc.vector.tensor_copy(out=dst, in_=accs[et])


            # --- per-n-block building blocks ---
            def emit_rcol(rgb_bf, nm):
                # rgbT columns for an n-block
                rcol = colp.tile([P, DC, NBLK], BF16, tag="col",
                                 name=f"rcol_{nm}")
                for dc in range(DC):
                    ps_t = psp.tile([P, NBLK], BF16, tag="ps",
                                    name=f"ps_t_{nm}_{dc}")
                    for s in range(SUB):
                        nc.tensor.transpose(
                            ps_t[:, s * P:(s + 1) * P],
                            rgb_bf[:, s, dc * P:(dc + 1) * P],
                            ident,
                        )
                    nc.vector.tensor_copy(out=rcol[:, dc, :], in_=ps_t)
                return rcol

            def emit_qproj(rcol, nm):
                qt = qtp.tile([P, DC, NBLK], BF16, tag="qt", name=f"qt_{nm}")
                for et in range(DC):
                    ps_q = psp.tile([P, NBLK], F32, tag="ps",
                                    name=f"ps_q_{nm}_{et}")
                    for dc in range(DC):
                        nc.tensor.matmul(
                            ps_q,
                            wq_bf[:, dc, et * P:(et + 1) * P],
                            rcol[:, dc, :],
                            start=(dc == 0),
                            stop=(dc == DC - 1),
                        )
                    if et % 2 == 0:
                        nc.scalar.copy(out=qt[:, et, :], in_=ps_q)
                    else:
                        nc.vector.tensor_copy(out=qt[:, et, :], in_=ps_q)
                return qt

            def emit_scores(qt, p_blk, mts, nm):
                # scoresT[m, nblk] -> P = exp(scoresT / 32)
                for mt in mts:
                    ps_s = psp.tile([P, NBLK], F32, tag="ps",
                                    name=f"ps_s_{nm}_{mt}")
                    for et in range(DC):
                        nc.tensor.matmul(
                            ps_s,
                            kt_bf[:, et, mt * P:(mt + 1) * P],
                            qt[:, et, :],
                            start=(et == 0),
                            stop=(et == DC - 1),
                        )
                    nc.scalar.activation(
                        out=p_blk[:, mt, :],
                        in_=ps_s,
                        func=mybir.ActivationFunctionType.Exp,
                        scale=1.0 / 32.0,
                    )

            # --- prologue PE pipeline: transposes of group mg+1 are emitted
            # before the kT matmuls of group mg, so the PE has transpose work
            # while Wk is still loading ---
            emit_ft(0)
            emit_ft(1)
            emit_kt(0)
            emit_ft(2)
            emit_kt(1)
            emit_ft(3)
            emit_kt(2)
            emit_kt(3)
            rcol0 = emit_rcol(rgb_bf0, 0)
            qt_cur = emit_qproj(rcol0, 0)

            for ng in range(NG):
                p_blk = pblkp.tile([P, NT, NBLK], BF16, tag="pblk",
                                   name=f"pblk_{ng}")
                emit_scores(qt_cur, p_blk, range(NT), ng)

                # prefetch + transpose + project the NEXT n-block's q before
                # the long U phase, so the PE never stalls at the boundary
                if ng + 1 < NG:
                    rgb_bf_next = load_rgb_group(ng + 1)[0]
                    rcol_next = emit_rcol(rgb_bf_next, ng + 1)
                    qt_cur = emit_qproj(rcol_next, ng + 1)

                # U[n, d] + colsum, then normalize and store
                for ntl in range(SUB):
                    n0 = ntl * P
                    ps_u = psup.tile([P, D], F32, tag="psu")
                    ps_cs = psp.tile([P, NBLK], F32, tag="ps")
                    for mc in range(NT):
                        lhs = p_blk[:, mc, n0:n0 + P]
                        nc.tensor.matmul(
                            ps_u[:, 0:NBLK], lhs, freq_bf[:, mc, 0:NBLK],
                            start=(mc == 0), stop=(mc == NT - 1),
                        )
                        nc.tensor.matmul(
                            ps_u[:, NBLK:D], lhs, freq_bf[:, mc, NBLK:D],
                            start=(mc == 0), stop=(mc == NT - 1),
                        )
                        nc.tensor.matmul(
                            ps_cs[:, 0:1], lhs, ones,
                            start=(mc == 0), stop=(mc == NT - 1),
                        )
                    rc = smallp.tile([P, 1], F32, tag="rc")
                    nc.vector.reciprocal(rc, ps_cs[:, 0:1])
                    ot = outp.tile([P, D], F32, tag="ot")
                    # out = (U * (1/colsum)) * 0.5   (fusion weight)
                    nc.vector.tensor_scalar(
                        out=ot, in0=ps_u, scalar1=rc, scalar2=0.5,
                        op0=mybir.AluOpType.mult, op1=mybir.AluOpType.mult,
                    )
                    row0 = ng * NBLK + n0
                    nc.sync.dma_start(out=out[row0:row0 + P, D:2 * D], in_=ot)

    _split_multi_waits(nc)
    return nc


_CACHE: dict = {}


def _get_program() -> bass.Bass:
    if "nc" not in _CACHE:
        _CACHE["nc"] = build_program()
    return _CACHE["nc"]


def _run(in_maps, trace=False, **kw):
    from concourse.bass_utils import run_bass_kernel_spmd

    nc = _get_program()
    return run_bass_kernel_spmd(nc, in_maps, list(range(B)), trace=trace, **kw)


def kernel(rgb, freq, ifreq=None, Wq=None, Wk=None, Wv=None, **_unused):
    rgb = np.asarray(rgb, dtype=np.float32)
    freq = np.asarray(freq, dtype=np.float32)
    Wq = np.ascontiguousarray(np.asarray(Wq, dtype=np.float32))
    Wk = np.ascontiguousarray(np.asarray(Wk, dtype=np.float32))
    in_maps = [
        {
            "rgb": np.ascontiguousarray(rgb[c]),
            "freq": np.ascontiguousarray(freq[c]),
            "Wq": Wq,
            "Wk": Wk,
        }
        for c in range(B)
    ]
    res = _run(in_maps, trace=False)
    return np.stack([res.results[c]["out"] for c in range(B)], axis=0)



# revision 10
# speedup vs baseline: 1.4077x; 1.4077x over previous
"""
Trainium2 Bass kernel for nn_CrossAttention_62027917689453 — fp8 DoubleRow.

Math (per batch b):
    q = rgb @ Wq                       (N, E)
    k = freq @ Wk                      (N, E)
    scores = q @ k.T / sqrt(E)         (N, N)
    attn = softmax(scores, axis=-1)
    attn_out = attn @ freq             (N, D)
    out = concat([rgb, 0.5 * attn_out], axis=-1)   (N, 2D)

(ifreq / Wv are dead inputs in the reference and are ignored.)

Sharding: data-parallel over batch — 8 batches onto 8 NeuronCores.

fp8 scheme (all matmuls float8e4 + perf_mode=DoubleRow, 2 fp8/PE cell,
contraction 256/instruction):
  - Wq/Wk are scaled by 32 on load so their entries are ~N(0,1) (raw
    entries ~N(0, 1/1024) would be subnormal in e4m3).  q', k' then have
    sigma=32 (max ~185 < 240 = TRN e4m3 max).  scores' = 1024 * raw.
  - exp uses scale=1/32768 and bias=-3: P = exp(scores/32 - 3).  The
    constant bias cancels in the softmax normalization and keeps
    max(P) ~ 31 < 240 so the fp8 store of P cannot overflow to Inf.
  - P is stored fp8; the softmax denominator is computed FROM THE SAME
    fp8 P values (ones-stationary DoubleRow matmuls accumulating a
    [1, 512] PSUM row), so numerator/denominator stay consistent.
  - The [1, nblk] column-sum row is moved into [n-partition, 1] layout
    with 4 tiny K=1 matmuls (vector outer product with scalar 1), after
    a reciprocal on DVE.  Normalization multiplies U by rc * 0.5.
  - Scores are computed TRANSPOSED (sT[m, n]) so exp(sT) is directly the
    stationary operand of the attention-output matmul, as in the bf16
    version.  All PE transposes (freqT / rgbT) run on fp8 data.
"""

import numpy as np

import concourse.bass as bass
import concourse.mybir as mybir
import concourse.tile as tile
from concourse.tile import TileContext

from concourse.masks import make_identity

F32 = mybir.dt.float32
F16 = mybir.dt.float16
F8 = mybir.dt.float8e4
DR = mybir.MatmulPerfMode.DoubleRow
EXP = mybir.ActivationFunctionType.Exp

B = 8          # batches == cores
N = 2048       # sequence length (n and m)
D = 1024       # feature dim (d and e)
P = 128        # partitions
NT = N // P    # 16  row chunks
DC = D // P    # 8   feature chunks
PAIR = DC // 2   # 4 DoubleRow contraction steps over d/e
MPAIR = NT // 2  # 8 DoubleRow contraction steps over m
NBLK = 512     # n-block width for the q/scores pipeline
NG = N // NBLK # 4   n-blocks
SUB = NBLK // P  # 4 row-chunks per n-block

WSCALE = 32.0              # Wq/Wk prescale (fp8 dynamic range)
EXP_SCALE = 1.0 / (WSCALE * WSCALE * 32.0)   # recovers scores/sqrt(E)
EXP_BIAS = -3.0            # constant shift, cancels in normalization


def _split_multi_waits(nc: bass.Bass) -> int:
    """The walrus build in this container cannot encode multi-semaphore waits
    on several instruction structs (CTRL Drain, PSEUDO_DMA_DIRECT2D, ...):
    setupSyncWait throws an internal error.  Rewrite every instruction that
    carries more than one wait so the extra waits sit on standalone
    single-wait EventSemaphore instructions immediately before it."""
    n_split = 0
    for f in nc.m.functions:
        for blk in f.blocks:
            insts = blk.instructions
            new: list = []
            changed = False
            for inst in insts:
                si = inst.sync_info
                if si is not None and len(si.on_wait) > 1:
                    waits = list(si.on_wait)
                    for w in waits[:-1]:
                        n_split += 1
                        ev = mybir.InstEventSemaphore(
                            name=f"I-msw-{n_split}",
                            ins=[],
                            outs=[],
                            sync_info=mybir.SyncInfo(on_wait=[w], on_update=[]),
                        )
                        ev.engine = inst.engine
                        new.append(ev)
                    si.on_wait.clear()
                    si.on_wait.append(waits[-1])
                    changed = True
                new.append(inst)
            if changed:
                insts[:] = new
    return n_split


def build_program() -> bass.Bass:
    nc = bass.Bass()
    rgb = nc.declare_dram_parameter("rgb", [N, D], F32, isOutput=False)
    freq = nc.declare_dram_parameter("freq", [N, D], F32, isOutput=False)
    wq = nc.declare_dram_parameter("Wq", [D, D], F32, isOutput=False)
    wk = nc.declare_dram_parameter("Wk", [D, D], F32, isOutput=False)
    out = nc.declare_dram_parameter("out", [N, 2 * D], F32, isOutput=True)

    with TileContext(nc) as tc:
        with (
            tc.tile_pool(name="statics", bufs=1) as statics,
            tc.tile_pool(name="ld", bufs=4) as ldp,
            tc.tile_pool(name="bfp", bufs=2) as bfp,
            tc.tile_pool(name="col", bufs=2) as colp,
            tc.tile_pool(name="qtp", bufs=2) as qtp,
            tc.tile_pool(name="pblk", bufs=2) as pblkp,
            tc.tile_pool(name="outp", bufs=3) as outp,
            tc.tile_pool(name="small", bufs=8) as smallp,
            tc.tile_pool(name="ps", bufs=4, space="PSUM") as psp,
            tc.tile_pool(name="psu", bufs=2, space="PSUM") as psup,
        ):
            ident = statics.tile([P, P], F8, tag="ident")
            make_identity(nc, ident)
            # colsum stationary: ones pair for DoubleRow.  The pair stride
            # (dim1) must be a multiple of 16 bytes, hence the padded shape.
            ones2_t = statics.tile([P, 2, 16], F8, tag="ones2")
            nc.vector.memset(ones2_t, 1.0)
            ones2 = ones2_t[:, :, 0:1]
            # K=1 transpose helper: [1, 1] of one
            ones1 = statics.tile([1, 1], F16, tag="ones1")
            nc.vector.memset(ones1, 1.0)
            # per-partition bias column for the exp activation
            ebias = statics.tile([P, 1], F32, tag="ebias")
            nc.vector.memset(ebias, EXP_BIAS)

            wq8 = statics.tile([P, DC, D], F8, tag="wq")
            wk8 = statics.tile([P, DC, D], F8, tag="wk")
            freq8 = statics.tile([P, NT, D], F8, tag="freq8")

            # DMA issue order is the critical-path order: the first PE work
            # (freqT transposes) needs the early freq chunks; kT needs Wk;
            # qT of block 0 needs rgb block 0 + Wq; remaining rgb blocks
            # stream inside the main loop.
            def load_freq(mc):
                t = ldp.tile([P, D], F32, tag="ld")
                nc.sync.dma_start(out=t, in_=freq[mc * P:(mc + 1) * P, :])
                # input casts on gpsimd: DVE/ScalarE stay free for the
                # PSUM-drain copies that gate the PE pipeline
                nc.gpsimd.tensor_copy(out=freq8[:, mc, :], in_=t)

            def load_wk(dc):
                t2 = ldp.tile([P, D], F32, tag="ld")
                nc.sync.dma_start(out=t2, in_=wk[dc * P:(dc + 1) * P, :])
                nc.vector.tensor_scalar_mul(wk8[:, dc, :], t2, WSCALE)

            def load_rgb_group(ng, defer_passthrough=False):
                # load rgb chunks; write the rgb passthrough output half
                rgb8 = bfp.tile([P, SUB, D], F8, tag="rgb8",
                                name=f"rgb8_{ng}")
                fp32_chunks = []
                for s in range(SUB):
                    nchunk = ng * SUB + s
                    t = ldp.tile([P, D], F32, tag="ld")
                    nc.sync.dma_start(
                        out=t, in_=rgb[nchunk * P:(nchunk + 1) * P, :]
                    )
                    nc.gpsimd.tensor_copy(out=rgb8[:, s, :], in_=t)
                    if defer_passthrough:
                        fp32_chunks.append(t)
                    else:
                        nc.sync.dma_start(
                            out=out[nchunk * P:(nchunk + 1) * P, 0:D], in_=t
                        )
                return rgb8, fp32_chunks

            for mc in range(4):
                load_freq(mc)
            for dc in range(DC):
                load_wk(dc)
            for mc in range(4, NT):
                load_freq(mc)
            rgb8_0, rgb0_chunks = load_rgb_group(0, defer_passthrough=True)
            for dc in range(DC):
                t = ldp.tile([P, D], F32, tag="ld")
                nc.sync.dma_start(out=t, in_=wq[dc * P:(dc + 1) * P, :])
                nc.vector.tensor_scalar_mul(wq8[:, dc, :], t, WSCALE)

            # ng=0 passthrough writes issue after the critical-path loads
            for s, t in enumerate(rgb0_chunks):
                nc.sync.dma_start(out=out[s * P:(s + 1) * P, 0:D], in_=t)

            # --- kT[e, m] = Wk'[d, e]^T  freqT[d, m]  (all m up front) ---
            kt8 = statics.tile([P, DC, N], F8, tag="kt")
            fcols = [None] * NG

            def emit_ft(mg):
                fcol = colp.tile([P, DC, NBLK], F8, tag="col")
                for dc in range(DC):
                    # fp8 transpose results land in 2-byte cells (walrus:
                    # "FP8 transpose mode must have output element step of
                    # 2"), so the PSUM staging tile is [P, NBLK, 2] and the
                    # drain copy reads the even bytes.
                    ps_t = psp.tile([P, NBLK, 2], F8, tag="ps")
                    for s in range(SUB):
                        mc = mg * SUB + s
                        nc.tensor.transpose(
                            ps_t[:, s * P:(s + 1) * P, 0],
                            freq8[:, mc, dc * P:(dc + 1) * P],
                            ident,
                        )
                    nc.vector.tensor_copy(out=fcol[:, dc, :],
                                          in_=ps_t[:, :, 0])
                fcols[mg] = fcol

            def emit_kt(mg):
                # pair-outer accumulation: all 8 PSUM banks hold one et-tile
                # accumulator each, so kT matmuls start as soon as the first
                # Wk pair is resident.
                fcol = fcols[mg]
                acc_a = psup.tile([P, D], F32, tag="psu")
                acc_b = psup.tile([P, D], F32, tag="psu")
                accs = [
                    acc_a[:, 0:NBLK], acc_a[:, NBLK:D],
                    acc_b[:, 0:NBLK], acc_b[:, NBLK:D],
                ] + [
                    psp.tile([P, NBLK], F32, tag="ps", name=f"kt_acc_{mg}_{j}")
                    for j in range(4)
                ]
                for c in range(PAIR):
                    for et in range(DC):
                        nc.tensor.matmul(
                            accs[et],
                            wk8[:, 2 * c:2 * c + 2, et * P:(et + 1) * P],
                            fcol[:, 2 * c:2 * c + 2, :],
                            start=(c == 0),
                            stop=(c == PAIR - 1),
                            perf_mode=DR,
                        )
                for et in range(DC):
                    dst = kt8[:, et, mg * NBLK:(mg + 1) * NBLK]
                    if et % 2 == 0:
                        nc.scalar.copy(out=dst, in_=accs[et])
                    else:
                        nc.vector.tensor_copy(out=dst, in_=accs[et])

            # --- per-n-block building blocks ---
            def emit_rcol(rgb8, nm):
                # rgbT columns for an n-block
                rcol = colp.tile([P, DC, NBLK], F8, tag="col",
                                 name=f"rcol_{nm}")
                for dc in range(DC):
                    ps_t = psp.tile([P, NBLK, 2], F8, tag="ps",
                                    name=f"ps_t_{nm}_{dc}")
                    for s in range(SUB):
                        nc.tensor.transpose(
                            ps_t[:, s * P:(s + 1) * P, 0],
                            rgb8[:, s, dc * P:(dc + 1) * P],
                            ident,
                        )
                    nc.vector.tensor_copy(out=rcol[:, dc, :],
                                          in_=ps_t[:, :, 0])
                return rcol

            def emit_qproj(rcol, nm):
                qt = qtp.tile([P, DC, NBLK], F8, tag="qt", name=f"qt_{nm}")
                for et in range(DC):
                    ps_q = psp.tile([P, NBLK], F32, tag="ps",
                                    name=f"ps_q_{nm}_{et}")
                    for c in range(PAIR):
                        nc.tensor.matmul(
                            ps_q,
                            wq8[:, 2 * c:2 * c + 2, et * P:(et + 1) * P],
                            rcol[:, 2 * c:2 * c + 2, :],
                            start=(c == 0),
                            stop=(c == PAIR - 1),
                            perf_mode=DR,
                        )
                    if et % 2 == 0:
                        nc.scalar.copy(out=qt[:, et, :], in_=ps_q)
                    else:
                        nc.vector.tensor_copy(out=qt[:, et, :], in_=ps_q)
                return qt

            def emit_scores(qt, p_blk, aux, nm):
                # scoresT[m, nblk] -> P = exp(scoresT/32768 - 3), fp8.
                # Column sums of P accumulate into aux[0:1, :] via
                # ones-stationary DoubleRow matmuls as pairs complete.
                for mt in range(NT):
                    ps_s = psp.tile([P, NBLK], F32, tag="ps",
                                    name=f"ps_s_{nm}_{mt}")
                    for c in range(PAIR):
                        nc.tensor.matmul(
                            ps_s,
                            kt8[:, 2 * c:2 * c + 2, mt * P:(mt + 1) * P],
                            qt[:, 2 * c:2 * c + 2, :],
                            start=(c == 0),
                            stop=(c == PAIR - 1),
                            perf_mode=DR,
                        )
                    nc.scalar.activation(
                        out=p_blk[:, mt, :],
                        in_=ps_s,
                        func=EXP,
                        scale=EXP_SCALE,
                        bias=ebias[:, 0:1],
                    )
                    if mt % 2 == 1:
                        c = mt // 2
                        nc.tensor.matmul(
                            aux[0:1, 0:NBLK],
                            ones2,
                            p_blk[:, mt - 1:mt + 1, :],
                            start=(c == 0),
                            stop=(c == MPAIR - 1),
                            perf_mode=DR,
                        )

            # --- prologue PE pipeline: transposes of group mg+1 are emitted
            # before the kT matmuls of group mg, so the PE has transpose work
            # while Wk is still loading ---
            emit_ft(0)
            emit_ft(1)
            emit_kt(0)
            emit_ft(2)
            emit_kt(1)
            emit_ft(3)
            emit_kt(2)
            emit_kt(3)
            rcol0 = emit_rcol(rgb8_0, 0)
            qt_cur = emit_qproj(rcol0, 0)

            for ng in range(NG):
                p_blk = pblkp.tile([P, NT, NBLK], F8, tag="pblk",
                                   name=f"pblk_{ng}")
                # aux bank: [0:1, 0:NBLK] holds the colsum row; cols
                # RCOL0..RCOL0+3 hold the transposed reciprocals
                RCOL0 = NBLK - SUB
                aux = psp.tile([P, NBLK], F32, tag="ps", name=f"aux_{ng}")
                emit_scores(qt_cur, p_blk, aux, ng)

                # prefetch + transpose + project the NEXT n-block's q before
                # the long U phase, so the PE never stalls at the boundary
                if ng + 1 < NG:
                    rgb8_next = load_rgb_group(ng + 1)[0]
                    rcol_next = emit_rcol(rgb8_next, ng + 1)
                    qt_cur = emit_qproj(rcol_next, ng + 1)

                # reciprocal of the colsum row, then transpose into
                # [n-partition, 1] layout via 4 K=1 matmuls
                rc_row = smallp.tile([1, NBLK], F16, tag="rcrow",
                                     name=f"rc_row_{ng}")
                with nc.allow_low_precision(
                    reason="fp16 reciprocal of O(100) colsums: 0.05% rel err, "
                    "well inside the output tolerance"
                ):
                    nc.vector.reciprocal(rc_row, aux[0:1, 0:NBLK])
                for ntl in range(SUB):
                    nc.tensor.matmul(
                        aux[:, RCOL0 + ntl:RCOL0 + ntl + 1],
                        rc_row[0:1, ntl * P:(ntl + 1) * P],
                        ones1,
                        start=True,
                        stop=True,
                    )
                rc_sb = smallp.tile([P, SUB], F32, tag="rcsb",
                                    name=f"rc_sb_{ng}")
                nc.scalar.copy(out=rc_sb, in_=aux[:, RCOL0:RCOL0 + SUB])

                # U[n, d]: DoubleRow accumulation over m pairs
                for ntl in range(SUB):
                    n0 = ntl * P
                    ps_u = psup.tile([P, D], F32, tag="psu")
                    for c in range(MPAIR):
                        lhs = p_blk[:, 2 * c:2 * c + 2, n0:n0 + P]
                        nc.tensor.matmul(
                            ps_u[:, 0:NBLK], lhs,
                            freq8[:, 2 * c:2 * c + 2, 0:NBLK],
                            start=(c == 0), stop=(c == MPAIR - 1),
                            perf_mode=DR,
                        )
                        nc.tensor.matmul(
                            ps_u[:, NBLK:D], lhs,
                            freq8[:, 2 * c:2 * c + 2, NBLK:D],
                            start=(c == 0), stop=(c == MPAIR - 1),
                            perf_mode=DR,
                        )
                    ot = outp.tile([P, D], F32, tag="ot")
                    # out = (U * (1/colsum)) * 0.5   (fusion weight)
                    nc.vector.tensor_scalar(
                        out=ot, in0=ps_u,
                        scalar1=rc_sb[:, ntl:ntl + 1], scalar2=0.5,
                        op0=mybir.AluOpType.mult, op1=mybir.AluOpType.mult,
                    )
                    row0 = ng * NBLK + n0
                    nc.sync.dma_start(out=out[row0:row0 + P, D:2 * D], in_=ot)

    _split_multi_waits(nc)
    return nc


_CACHE: dict = {}


def _get_program() -> bass.Bass:
    if "nc" not in _CACHE:
        _CACHE["nc"] = build_program()
    return _CACHE["nc"]


def _run(in_maps, trace=False, **kw):
    from concourse.bass_utils import run_bass_kernel_spmd

    nc = _get_program()
    return run_bass_kernel_spmd(nc, in_maps, list(range(B)), trace=trace, **kw)


def kernel(rgb, freq, ifreq=None, Wq=None, Wk=None, Wv=None, **_unused):
    rgb = np.asarray(rgb, dtype=np.float32)
    freq = np.asarray(freq, dtype=np.float32)
    Wq = np.ascontiguousarray(np.asarray(Wq, dtype=np.float32))
    Wk = np.ascontiguousarray(np.asarray(Wk, dtype=np.float32))
    in_maps = [
        {
            "rgb": np.ascontiguousarray(rgb[c]),
            "freq": np.ascontiguousarray(freq[c]),
            "Wq": Wq,
            "Wk": Wk,
        }
        for c in range(B)
    ]
    res = _run(in_maps, trace=False)
    return np.stack([res.results[c]["out"] for c in range(B)], axis=0)


# revision 13
# speedup vs baseline: 1.5994x; 1.1361x over previous
"""
Trainium2 Bass kernel for nn_CrossAttention_62027917689453 — fp8 DoubleRow.

Math (per batch b):
    q = rgb @ Wq                       (N, E)
    k = freq @ Wk                      (N, E)
    scores = q @ k.T / sqrt(E)         (N, N)
    attn = softmax(scores, axis=-1)
    attn_out = attn @ freq             (N, D)
    out = concat([rgb, 0.5 * attn_out], axis=-1)   (N, 2D)

(ifreq / Wv are dead inputs in the reference and are ignored.)

Sharding: data-parallel over batch — 8 batches onto 8 NeuronCores.

fp8 scheme (all matmuls float8e4 + perf_mode=DoubleRow, 2 fp8/PE cell,
contraction 256/instruction):
  - Wq/Wk are scaled by 32 on load so their entries are ~N(0,1) (raw
    entries ~N(0, 1/1024) would be subnormal in e4m3).  q', k' then have
    sigma=32 (max ~185 < 240 = TRN e4m3 max).  scores' = 1024 * raw.
  - exp uses scale=1/32768 and bias=-3: P = exp(scores/32 - 3).  The
    constant bias cancels in the softmax normalization and keeps
    max(P) ~ 31 < 240 so the fp8 store of P cannot overflow to Inf.
  - P is stored fp8; the softmax denominator is computed FROM THE SAME
    fp8 P values (ones-stationary DoubleRow matmuls accumulating a
    [1, 512] PSUM row), so numerator/denominator stay consistent.
  - The [1, nblk] column-sum row is moved into [n-partition, 1] layout
    with 4 tiny K=1 matmuls (vector outer product with scalar 1), after
    a reciprocal on DVE.  Normalization multiplies U by rc * 0.5.
  - Scores are computed TRANSPOSED (sT[m, n]) so exp(sT) is directly the
    stationary operand of the attention-output matmul, as in the bf16
    version.  All PE transposes (freqT / rgbT) run on fp8 data.
"""

import numpy as np

import concourse.bass as bass
import concourse.mybir as mybir
import concourse.tile as tile
from concourse.tile import TileContext

from concourse.masks import make_identity

F32 = mybir.dt.float32
F16 = mybir.dt.float16
F8 = mybir.dt.float8e4
DR = mybir.MatmulPerfMode.DoubleRow
EXP = mybir.ActivationFunctionType.Exp

B = 8          # batches == cores
N = 2048       # sequence length (n and m)
D = 1024       # feature dim (d and e)
P = 128        # partitions
NT = N // P    # 16  row chunks
DC = D // P    # 8   feature chunks
PAIR = DC // 2   # 4 DoubleRow contraction steps over d/e
MPAIR = NT // 2  # 8 DoubleRow contraction steps over m
NBLK = 512     # n-block width for the q/scores pipeline
NG = N // NBLK # 4   n-blocks
SUB = NBLK // P  # 4 row-chunks per n-block

WSCALE = 32.0              # Wq/Wk prescale (fp8 dynamic range)
EXP_SCALE = 1.0 / (WSCALE * WSCALE * 32.0)   # recovers scores/sqrt(E)
EXP_BIAS = -3.0            # constant shift, cancels in normalization


def _split_multi_waits(nc: bass.Bass) -> int:
    """The walrus build in this container cannot encode multi-semaphore waits
    on several instruction structs (CTRL Drain, PSEUDO_DMA_DIRECT2D, ...):
    setupSyncWait throws an internal error.  Rewrite every instruction that
    carries more than one wait so the extra waits sit on standalone
    single-wait EventSemaphore instructions immediately before it."""
    n_split = 0
    for f in nc.m.functions:
        for blk in f.blocks:
            insts = blk.instructions
            new: list = []
            changed = False
            for inst in insts:
                si = inst.sync_info
                if si is not None and len(si.on_wait) > 1:
                    waits = list(si.on_wait)
                    for w in waits[:-1]:
                        n_split += 1
                        ev = mybir.InstEventSemaphore(
                            name=f"I-msw-{n_split}",
                            ins=[],
                            outs=[],
                            sync_info=mybir.SyncInfo(on_wait=[w], on_update=[]),
                        )
                        ev.engine = inst.engine
                        new.append(ev)
                    si.on_wait.clear()
                    si.on_wait.append(waits[-1])
                    changed = True
                new.append(inst)
            if changed:
                insts[:] = new
    return n_split


def build_program() -> bass.Bass:
    nc = bass.Bass()
    rgb = nc.declare_dram_parameter("rgb", [N, D], F32, isOutput=False)
    freq = nc.declare_dram_parameter("freq", [N, D], F32, isOutput=False)
    wq = nc.declare_dram_parameter("Wq", [D, D], F32, isOutput=False)
    wk = nc.declare_dram_parameter("Wk", [D, D], F32, isOutput=False)
    out = nc.declare_dram_parameter("out", [N, 2 * D], F32, isOutput=True)

    with TileContext(nc) as tc:
        with (
            tc.tile_pool(name="statics", bufs=1) as statics,
            tc.tile_pool(name="ld", bufs=4) as ldp,
            tc.tile_pool(name="bfp", bufs=2) as bfp,
            tc.tile_pool(name="col", bufs=2) as colp,
            tc.tile_pool(name="qtp", bufs=2) as qtp,
            tc.tile_pool(name="pblk", bufs=2) as pblkp,
            tc.tile_pool(name="outp", bufs=3) as outp,
            tc.tile_pool(name="small", bufs=8) as smallp,
            tc.tile_pool(name="ps", bufs=4, space="PSUM") as psp,
            tc.tile_pool(name="psu", bufs=2, space="PSUM") as psup,
        ):
            ident = statics.tile([P, P], F8, tag="ident")
            make_identity(nc, ident)
            # colsum stationary: ones pair for DoubleRow.  The pair stride
            # (dim1) must be a multiple of 16 bytes, hence the padded shape.
            ones2_t = statics.tile([P, 2, 16], F8, tag="ones2")
            nc.vector.memset(ones2_t, 1.0)
            ones2 = ones2_t[:, :, 0:1]
            # K=1 transpose helper: [1, 1] of one
            ones1 = statics.tile([1, 1], F16, tag="ones1")
            nc.vector.memset(ones1, 1.0)
            # per-partition bias column for the exp activation
            ebias = statics.tile([P, 1], F32, tag="ebias")
            nc.vector.memset(ebias, EXP_BIAS)

            wq8 = statics.tile([P, DC, D], F8, tag="wq")
            wk8 = statics.tile([P, DC, D], F8, tag="wk")
            freq8 = statics.tile([P, NT, D], F8, tag="freq8")

            # DMA issue order is the critical-path order: the first PE work
            # (freqT transposes) needs the early freq chunks; kT needs Wk;
            # qT of block 0 needs rgb block 0 + Wq; remaining rgb blocks
            # stream inside the main loop.
            # Input casts are engine-balanced: gpsimd's CAST is ~5x slower
            # than DVE/ScalarE, so it only gets the non-critical prefetched
            # rgb blocks (ng>=1); everything on the prologue critical path
            # alternates between vector and scalar.
            def load_freq(mc):
                t = ldp.tile([P, D], F32, tag="ld")
                nc.sync.dma_start(out=t, in_=freq[mc * P:(mc + 1) * P, :])
                if mc % 2 == 0:
                    nc.vector.tensor_copy(out=freq8[:, mc, :], in_=t)
                else:
                    nc.scalar.copy(out=freq8[:, mc, :], in_=t)

            def load_w(dram, dst, dc):
                t2 = ldp.tile([P, D], F32, tag="ld")
                nc.sync.dma_start(out=t2, in_=dram[dc * P:(dc + 1) * P, :])
                if dc % 2 == 0:
                    nc.vector.tensor_scalar_mul(dst[:, dc, :], t2, WSCALE)
                else:
                    nc.scalar.activation(
                        out=dst[:, dc, :], in_=t2,
                        func=mybir.ActivationFunctionType.Copy, scale=WSCALE,
                    )

            def load_rgb_group(ng, defer_passthrough=False):
                # load rgb chunks; write the rgb passthrough output half
                rgb8 = bfp.tile([P, SUB, D], F8, tag="rgb8",
                                name=f"rgb8_{ng}")
                fp32_chunks = []
                for s in range(SUB):
                    nchunk = ng * SUB + s
                    t = ldp.tile([P, D], F32, tag="ld")
                    nc.sync.dma_start(
                        out=t, in_=rgb[nchunk * P:(nchunk + 1) * P, :]
                    )
                    if ng == 0:
                        nc.vector.tensor_copy(out=rgb8[:, s, :], in_=t)
                    else:
                        nc.gpsimd.tensor_copy(out=rgb8[:, s, :], in_=t)
                    if defer_passthrough:
                        fp32_chunks.append(t)
                    else:
                        nc.sync.dma_start(
                            out=out[nchunk * P:(nchunk + 1) * P, 0:D], in_=t
                        )
                return rgb8, fp32_chunks

            for mc in range(4):
                load_freq(mc)
            for dc in range(DC):
                load_w(wk, wk8, dc)
            for mc in range(4, NT):
                load_freq(mc)
            rgb8_0, rgb0_chunks = load_rgb_group(0, defer_passthrough=True)
            for dc in range(DC):
                load_w(wq, wq8, dc)

            # ng=0 passthrough writes issue after the critical-path loads
            for s, t in enumerate(rgb0_chunks):
                nc.sync.dma_start(out=out[s * P:(s + 1) * P, 0:D], in_=t)

            # --- kT[e, m] = Wk'[d, e]^T  freqT[d, m]  (all m up front) ---
            kt8 = statics.tile([P, DC, N], F8, tag="kt")
            fcols = [None] * NG

            def emit_ft(mg):
                fcol = colp.tile([P, DC, NBLK], F8, tag="col")
                for dc in range(DC):
                    # fp8 transpose results land in 2-byte cells (walrus:
                    # "FP8 transpose mode must have output element step of
                    # 2"), so the PSUM staging tile is [P, NBLK, 2] and the
                    # drain copy reads the even bytes.
                    ps_t = psp.tile([P, NBLK, 2], F8, tag="ps")
                    for s in range(SUB):
                        mc = mg * SUB + s
                        nc.tensor.transpose(
                            ps_t[:, s * P:(s + 1) * P, 0],
                            freq8[:, mc, dc * P:(dc + 1) * P],
                            ident,
                        )
                    nc.vector.tensor_copy(out=fcol[:, dc, :],
                                          in_=ps_t[:, :, 0])
                fcols[mg] = fcol

            def emit_kt(mg):
                # pair-outer accumulation: all 8 PSUM banks hold one et-tile
                # accumulator each, so kT matmuls start as soon as the first
                # Wk pair is resident.
                fcol = fcols[mg]
                acc_a = psup.tile([P, D], F32, tag="psu")
                acc_b = psup.tile([P, D], F32, tag="psu")
                accs = [
                    acc_a[:, 0:NBLK], acc_a[:, NBLK:D],
                    acc_b[:, 0:NBLK], acc_b[:, NBLK:D],
                ] + [
                    psp.tile([P, NBLK], F32, tag="ps", name=f"kt_acc_{mg}_{j}")
                    for j in range(4)
                ]
                for c in range(PAIR):
                    for et in range(DC):
                        nc.tensor.matmul(
                            accs[et],
                            wk8[:, 2 * c:2 * c + 2, et * P:(et + 1) * P],
                            fcol[:, 2 * c:2 * c + 2, :],
                            start=(c == 0),
                            stop=(c == PAIR - 1),
                            perf_mode=DR,
                        )
                for et in range(DC):
                    dst = kt8[:, et, mg * NBLK:(mg + 1) * NBLK]
                    if et % 2 == 0:
                        nc.scalar.copy(out=dst, in_=accs[et])
                    else:
                        nc.vector.tensor_copy(out=dst, in_=accs[et])

            # --- per-n-block building blocks ---
            def emit_rcol(rgb8, nm):
                # rgbT columns for an n-block
                rcol = colp.tile([P, DC, NBLK], F8, tag="col",
                                 name=f"rcol_{nm}")
                for dc in range(DC):
                    ps_t = psp.tile([P, NBLK, 2], F8, tag="ps",
                                    name=f"ps_t_{nm}_{dc}")
                    for s in range(SUB):
                        nc.tensor.transpose(
                            ps_t[:, s * P:(s + 1) * P, 0],
                            rgb8[:, s, dc * P:(dc + 1) * P],
                            ident,
                        )
                    nc.vector.tensor_copy(out=rcol[:, dc, :],
                                          in_=ps_t[:, :, 0])
                return rcol

            def emit_qproj(rcol, nm):
                qt = qtp.tile([P, DC, NBLK], F8, tag="qt", name=f"qt_{nm}")
                for et in range(DC):
                    ps_q = psp.tile([P, NBLK], F32, tag="ps",
                                    name=f"ps_q_{nm}_{et}")
                    for c in range(PAIR):
                        nc.tensor.matmul(
                            ps_q,
                            wq8[:, 2 * c:2 * c + 2, et * P:(et + 1) * P],
                            rcol[:, 2 * c:2 * c + 2, :],
                            start=(c == 0),
                            stop=(c == PAIR - 1),
                            perf_mode=DR,
                        )
                    if et % 2 == 0:
                        nc.scalar.copy(out=qt[:, et, :], in_=ps_q)
                    else:
                        nc.vector.tensor_copy(out=qt[:, et, :], in_=ps_q)
                return qt

            def emit_scores(qt, p_blk, aux, nm):
                # scoresT[m, nblk] -> P = exp(scoresT/32768 - 3), fp8.
                # Column sums of P accumulate into aux[0:1, :] via
                # ones-stationary DoubleRow matmuls as pairs complete.
                for mt in range(NT):
                    ps_s = psp.tile([P, NBLK], F32, tag="ps",
                                    name=f"ps_s_{nm}_{mt}")
                    for c in range(PAIR):
                        nc.tensor.matmul(
                            ps_s,
                            kt8[:, 2 * c:2 * c + 2, mt * P:(mt + 1) * P],
                            qt[:, 2 * c:2 * c + 2, :],
                            start=(c == 0),
                            stop=(c == PAIR - 1),
                            perf_mode=DR,
                        )
                    nc.scalar.activation(
                        out=p_blk[:, mt, :],
                        in_=ps_s,
                        func=EXP,
                        scale=EXP_SCALE,
                        bias=ebias[:, 0:1],
                    )
                    if mt % 2 == 1:
                        c = mt // 2
                        nc.tensor.matmul(
                            aux[0:1, 0:NBLK],
                            ones2,
                            p_blk[:, mt - 1:mt + 1, :],
                            start=(c == 0),
                            stop=(c == MPAIR - 1),
                            perf_mode=DR,
                        )

            # --- prologue PE pipeline: transposes of group mg+1 are emitted
            # before the kT matmuls of group mg, so the PE has transpose work
            # while Wk is still loading ---
            emit_ft(0)
            emit_ft(1)
            emit_kt(0)
            emit_ft(2)
            emit_kt(1)
            emit_ft(3)
            emit_kt(2)
            emit_kt(3)
            rcol0 = emit_rcol(rgb8_0, 0)
            qt_cur = emit_qproj(rcol0, 0)

            for ng in range(NG):
                p_blk = pblkp.tile([P, NT, NBLK], F8, tag="pblk",
                                   name=f"pblk_{ng}")
                # aux bank: [0:1, 0:NBLK] holds the colsum row; cols
                # RCOL0..RCOL0+3 hold the transposed reciprocals
                RCOL0 = NBLK - SUB
                aux = psp.tile([P, NBLK], F32, tag="ps", name=f"aux_{ng}")
                emit_scores(qt_cur, p_blk, aux, ng)

                # prefetch + transpose + project the NEXT n-block's q before
                # the long U phase, so the PE never stalls at the boundary
                if ng + 1 < NG:
                    rgb8_next = load_rgb_group(ng + 1)[0]
                    rcol_next = emit_rcol(rgb8_next, ng + 1)
                    qt_cur = emit_qproj(rcol_next, ng + 1)

                # transpose the colsum row into [n-partition, 1] layout via
                # 4 K=1 matmuls (fp16), THEN take the reciprocal in [P, SUB]
                # layout — a [1, 512] reciprocal runs on a single DVE lane
                # (~3.3us); the transposed [128, 4] one is ~100ns.
                cs_row = smallp.tile([1, NBLK], F16, tag="csrow",
                                     name=f"cs_row_{ng}")
                nc.scalar.copy(out=cs_row, in_=aux[0:1, 0:NBLK])
                for ntl in range(SUB):
                    nc.tensor.matmul(
                        aux[:, RCOL0 + ntl:RCOL0 + ntl + 1],
                        cs_row[0:1, ntl * P:(ntl + 1) * P],
                        ones1,
                        start=True,
                        stop=True,
                    )
                rc_sb = smallp.tile([P, SUB], F32, tag="rcsb",
                                    name=f"rc_sb_{ng}")
                nc.vector.reciprocal(rc_sb, aux[:, RCOL0:RCOL0 + SUB])

                # U[n, d]: DoubleRow accumulation over m pairs
                for ntl in range(SUB):
                    n0 = ntl * P
                    ps_u = psup.tile([P, D], F32, tag="psu")
                    for c in range(MPAIR):
                        lhs = p_blk[:, 2 * c:2 * c + 2, n0:n0 + P]
                        nc.tensor.matmul(
                            ps_u[:, 0:NBLK], lhs,
                            freq8[:, 2 * c:2 * c + 2, 0:NBLK],
                            start=(c == 0), stop=(c == MPAIR - 1),
                            perf_mode=DR,
                        )
                        nc.tensor.matmul(
                            ps_u[:, NBLK:D], lhs,
                            freq8[:, 2 * c:2 * c + 2, NBLK:D],
                            start=(c == 0), stop=(c == MPAIR - 1),
                            perf_mode=DR,
                        )
                    ot = outp.tile([P, D], F32, tag="ot")
                    # out = (U * (1/colsum)) * 0.5   (fusion weight)
                    nc.vector.tensor_scalar(
                        out=ot, in0=ps_u,
                        scalar1=rc_sb[:, ntl:ntl + 1], scalar2=0.5,
                        op0=mybir.AluOpType.mult, op1=mybir.AluOpType.mult,
                    )
                    row0 = ng * NBLK + n0
                    nc.sync.dma_start(out=out[row0:row0 + P, D:2 * D], in_=ot)

    _split_multi_waits(nc)
    return nc


_CACHE: dict = {}


def _get_program() -> bass.Bass:
    if "nc" not in _CACHE:
        _CACHE["nc"] = build_program()
    return _CACHE["nc"]


def _run(in_maps, trace=False, **kw):
    from concourse.bass_utils import run_bass_kernel_spmd

    nc = _get_program()
    return run_bass_kernel_spmd(nc, in_maps, list(range(B)), trace=trace, **kw)


def kernel(rgb, freq, ifreq=None, Wq=None, Wk=None, Wv=None, **_unused):
    rgb = np.asarray(rgb, dtype=np.float32)
    freq = np.asarray(freq, dtype=np.float32)
    Wq = np.ascontiguousarray(np.asarray(Wq, dtype=np.float32))
    Wk = np.ascontiguousarray(np.asarray(Wk, dtype=np.float32))
    in_maps = [
        {
            "rgb": np.ascontiguousarray(rgb[c]),
            "freq": np.ascontiguousarray(freq[c]),
            "Wq": Wq,
            "Wk": Wk,
        }
        for c in range(B)
    ]
    res = _run(in_maps, trace=False)
    return np.stack([res.results[c]["out"] for c in range(B)], axis=0)
